# revision 39
# baseline (speedup 1.0000x reference)
"""CorrLookup Trainium2 kernel (fine-grained merged-record dma_gather).

Reference op (RAFT-style 1-D correlation pyramid lookup): for each pixel n
(N = B*H*W = 196608) and level i (row width Wi = 256 >> i), sample the
pixel's correlation row at x = disp[n]/2^i + k, k = -4..4, with 1-D linear
interpolation and zeros padding; output (B, 36, H, W).

Design: one 256-B record per (pixel, u) with u = floor(disp)>>2 holds the
four level slices SLC = [13, 11, 10, 10] at OFF; with t = floor(disp),
t>>1 = 2u + d1 and t>>3 = u>>1 exactly, so levels 2/3 are pure 2-tap lerps
and levels 0/1 need 5/3 hat taps (h_j = relu(1 - |a - j|),
a_l = disp/2^l - (4>>l)*u).

The gather is dma_gather (indices int16, relative to a per-call base:
blocks of 512 pixels * 64 records = 32768 rows = the int16 range); it is
descriptor-bound (~2.3 ns/descriptor aggregate over 16 SDMA engines,
~57 us for 24576 descriptors), so everything else is pipelined under it:
  * index chunks -> gathers start ~7 us into the kernel,
  * static iota tables (index base, tap index field) are host constants,
  * per piece (3x48 + 36 + 12 cols): records are repacked to (slot, col)
    on the Act engine (DVE for the final sliver), interp MACs run on DVE
    with every operand packed step-1 f16 (2x perf mode; a broadcast on
    the last dim would force 1x), output DMA'd into contiguous HBM slabs.
Measured: 163.3 us (v1 baseline) -> 103.8 us, rel err 1.5e-3.
floor() is int-cast(x - 0.5) (round-to-nearest; ROBUST_FLOOR restores the
cast-agnostic 5-op floor).
Sharding: data-parallel over pixels; core c takes batch b = c.
"""

import numpy as np

P = 128
B, H, W = 8, 96, 256
NLVL = 4
K = 9                    # taps per level
NREC = 64                # records per pixel (u = t>>2)
REC = 128                # record slots (fp16) = 256 B
OFF = [0, 13, 24, 34]    # level slice offsets inside a record
SLC = [13, 11, 10, 10]   # level slice widths
TAPS = [5, 3, 2, 2]      # hat taps per level
WS = [W >> i for i in range(NLVL)]
NQ = 4                   # interp quarters

ROBUST_FLOOR = False     # True: cast-rounding-agnostic floor (more DVE ops)


def build_bass(n_pix=B * H * W // 8):
    """Single-core SPMD program.
    Inputs: rec [n_pix*NREC, REC] f16, disp_cm [P, n_pix/P] f32 (column-major
    pixels: n = col*128 + p), disp_w [P, n_pix/16] f32 (wrapped+replicated:
    disp_w[c, m] = disp[16*m + c%16]), base16 [P, mw/4] i16 (static gather
    index base), jf16 [P, 5, tcol] f16 (static tap index field).
    Output: outd [NQ, P, 36, tcol/NQ] f16 (host maps n = col*128 + p)."""
    import concourse.bass as bass
    import concourse.bacc as bacc
    import concourse.mybir as mybir
    from concourse.tile import TileContext

    f32 = mybir.dt.float32
    f16 = mybir.dt.float16
    i32 = mybir.dt.int32
    i16 = mybir.dt.int16
    Alu = mybir.AluOpType

    tcol = n_pix // P            # 192 pixel columns
    mw = n_pix // 16             # 1536 wrapped cols
    nblk = n_pix // 512          # 48 gather blocks (512 pixels each)
    nchunk = 4                   # index-computation chunks
    mwc = mw // nchunk           # 384 wrapped cols per chunk
    bpc = nblk // nchunk         # 12 gather blocks per chunk
    qcol = tcol // NQ            # 48 pixel columns per interp quarter
    USED = OFF[-1] + SLC[-1]     # 44 used record slots

    nc = bacc.Bacc(num_swdge_queues=4)
    rec = nc.declare_dram_parameter("rec", [n_pix * NREC, REC], f16, isOutput=False)
    disp_cm = nc.declare_dram_parameter("disp_cm", [P, tcol], f32, isOutput=False)
    disp_w = nc.declare_dram_parameter("disp_w", [P, mw], f32, isOutput=False)
    base16d = nc.declare_dram_parameter("base16", [P, mwc], i16, isOutput=False)
    jf16d = nc.declare_dram_parameter("jf16", [P, TAPS[0], tcol], f16,
                                      isOutput=False)
    outd = nc.declare_dram_parameter("outd", [P, NLVL * K * tcol], f16,
                                     isOutput=True)

    def robust_floor(pool, d_t, cols, tagp):
        """floor for d >= 0, any f32->i32 rounding mode. Returns i32."""
        fi = pool.tile([P, cols], i32, tag=tagp + "fi")
        nc.vector.tensor_copy(out=fi[:], in_=d_t[:])
        ff = pool.tile([P, cols], f32, tag=tagp + "ff")
        nc.vector.tensor_copy(out=ff[:], in_=fi[:])
        er = pool.tile([P, cols], f32, tag=tagp + "er")
        nc.vector.tensor_tensor(out=er[:], in0=d_t[:], in1=ff[:], op=Alu.subtract)
        ng = pool.tile([P, cols], i32, tag=tagp + "ng")
        nc.vector.tensor_scalar(out=ng[:], in0=er[:], scalar1=0.0, scalar2=None,
                                op0=Alu.is_lt)
        nc.vector.tensor_tensor(out=fi[:], in0=fi[:], in1=ng[:], op=Alu.subtract)
        return fi

    with TileContext(nc) as tc:
        with (
            tc.tile_pool(name="keep", bufs=1) as kp,
            tc.tile_pool(name="work", bufs=2) as wp,
            tc.tile_pool(name="res", bufs=2) as rp,
        ):
            # interp pieces: the last quarter is split into 12-col slivers
            # so each repack+MAC pipeline starts as soon as its gather
            # blocks land; only one sliver remains after the final gather
            PIECES = [(0, 48), (48, 48), (96, 48), (144, 12), (156, 12),
                      (168, 12), (180, 12)]
            assert sum(w for _, w in PIECES) == tcol

            g_full = kp.tile([P, tcol, REC], f16)

            base16 = kp.tile([P, mwc], i16)
            nc.sync.dma_start(out=base16[:], in_=base16d[:])
            # non-critical constants ride the Act HWDGE queue
            jf16 = kp.tile([P, TAPS[0], tcol], f16)
            nc.scalar.dma_start(out=jf16[:], in_=jf16d[:])

            # ---- wrapped int16 record indices, chunked -------------------
            # All index math and hat weights run BEFORE the first gather:
            # f32 DVE ops are 2-port-eligible and fight the SWDGE
            # descriptor rings for SBUF ports if run during the gather
            # phase (measured 5-8x inflation).
            CHUNKS = [(0, 12), (12, 12), (24, 12), (36, 12)]
            r16s = {}
            for b0, nb in CHUNKS:
                sl = slice(b0 * 32, (b0 + nb) * 32)
                cw = nb * 32
                dw = wp.tile([P, cw], f32, tag=f"dw{nb}")
                nc.sync.dma_start(out=dw[:], in_=disp_w[:, sl])
                uw16 = wp.tile([P, cw], i16, tag=f"uw{nb}")
                if ROBUST_FLOOR:
                    d4 = wp.tile([P, cw], f32, tag=f"d4w{nb}")
                    nc.vector.tensor_scalar(out=d4[:], in0=dw[:], scalar1=0.25,
                                            scalar2=None, op0=Alu.mult)
                    fi = robust_floor(wp, d4, cw, f"w{nb}")
                    nc.vector.tensor_copy(out=uw16[:], in_=fi[:])
                else:
                    # u = nearest_int(disp/4 - 0.5) == floor(disp/4) away
                    # from exact integers; at exact integers it may round
                    # low, which the taps cover (a hits its closed upper
                    # bound with zero lerp fraction).
                    d4 = wp.tile([P, cw], f32, tag=f"d4w{nb}")
                    nc.vector.tensor_scalar(out=d4[:], in0=dw[:], scalar1=0.25,
                                            scalar2=-0.5, op0=Alu.mult,
                                            op1=Alu.add)
                    nc.vector.tensor_copy(out=uw16[:], in_=d4[:])
                # base16 repeats every 32 cols and chunks are block-aligned
                r16 = kp.tile([P, cw], i16, tag=f"r{b0}")
                nc.vector.tensor_tensor(out=r16[:], in0=uw16[:],
                                        in1=base16[:, 0:cw], op=Alu.add)
                for bg in range(nb):
                    r16s[b0 + bg] = r16[:, 32 * bg : 32 * (bg + 1)]

            for g in range(nblk):
                nc.gpsimd.dma_gather(
                    out_ap=g_full[:, 4 * g : 4 * (g + 1), :],
                    in_ap=rec[32768 * g : 32768 * (g + 1), :],
                    idxs_ap=r16s[g],
                    num_idxs=512, num_idxs_reg=512, elem_size=REC,
                    single_packet=True, queue_num=g % 4,
                )

            # ---- per-pixel params (pixel layout) -------------------------
            disp_t = kp.tile([P, tcol], f32)
            nc.scalar.dma_start(out=disp_t[:], in_=disp_cm[:])

            d4c = wp.tile([P, tcol], f32, tag="d4c")
            if ROBUST_FLOOR:
                nc.vector.tensor_scalar(out=d4c[:], in0=disp_t[:], scalar1=0.25,
                                        scalar2=None, op0=Alu.mult)
                u_i = robust_floor(wp, d4c, tcol, "c")
            else:
                nc.vector.tensor_scalar(out=d4c[:], in0=disp_t[:], scalar1=0.25,
                                        scalar2=-0.5, op0=Alu.mult, op1=Alu.add)
                u_i = wp.tile([P, tcol], i32, tag="ui")
                nc.vector.tensor_copy(out=u_i[:], in_=d4c[:])
            u_f = kp.tile([P, tcol], f32)
            nc.vector.tensor_copy(out=u_f[:], in_=u_i[:])
            uh_i = wp.tile([P, tcol], i32, tag="uh")
            nc.vector.tensor_scalar(out=uh_i[:], in0=u_i[:], scalar1=1,
                                    scalar2=None, op0=Alu.logical_shift_right)
            uh_f = kp.tile([P, tcol], f32)
            nc.vector.tensor_copy(out=uh_f[:], in_=uh_i[:])

            # a_lvl = disp/2^lvl - (4>>lvl)*u   (lvl 3: disp/8 - (u>>1))
            a16s = []
            for lvl in range(NLVL):
                dl = wp.tile([P, tcol], f32, tag="dl")
                nc.scalar.mul(dl[:], disp_t[:], 1.0 / (1 << lvl))
                us = wp.tile([P, tcol], f32, tag="us")
                if lvl == 3:
                    nc.vector.tensor_tensor(out=us[:], in0=dl[:], in1=uh_f[:],
                                            op=Alu.subtract)
                else:
                    sc = wp.tile([P, tcol], f32, tag="sc")
                    nc.vector.tensor_scalar(out=sc[:], in0=u_f[:],
                                            scalar1=float(4 >> lvl),
                                            scalar2=None, op0=Alu.mult)
                    nc.vector.tensor_tensor(out=us[:], in0=dl[:], in1=sc[:],
                                            op=Alu.subtract)
                a16 = kp.tile([P, tcol], f16, tag=f"a16_{lvl}")
                nc.vector.tensor_copy(out=a16[:], in_=us[:])
                a16s.append(a16)

            # hat weights for levels 0/1: h_j = relu(1 - |a - j|), fp16
            h_ts = []
            for lvl in range(2):
                taps = TAPS[lvl]
                h_t = kp.tile([P, taps, tcol], f16, tag=f"h{lvl}")
                nc.vector.tensor_tensor(
                    out=h_t[:],
                    in0=a16s[lvl][:, None, :].to_broadcast([P, taps, tcol]),
                    in1=jf16[:, 0:taps, :],
                    op=Alu.subtract)
                hv = wp.tile([P, taps, tcol], f16, tag="hv")
                nc.vector.tensor_scalar(out=hv[:], in0=h_t[:], scalar1=-1.0,
                                        scalar2=1.0, op0=Alu.mult, op1=Alu.add)
                nc.vector.tensor_scalar(out=h_t[:], in0=h_t[:], scalar1=1.0,
                                        scalar2=None, op0=Alu.add)
                nc.vector.tensor_tensor(out=h_t[:], in0=h_t[:], in1=hv[:],
                                        op=Alu.min)
                nc.vector.tensor_scalar(out=h_t[:], in0=h_t[:], scalar1=0.0,
                                        scalar2=None, op0=Alu.max)
                h_ts.append(h_t)
            # levels 2/3: pure lerp, weights (1-a, a)
            fbar16s = []
            for lvl in (2, 3):
                fb = kp.tile([P, tcol], f16, tag=f"fb{lvl}")
                nc.vector.tensor_scalar(out=fb[:], in0=a16s[lvl][:],
                                        scalar1=-1.0, scalar2=1.0,
                                        op0=Alu.mult, op1=Alu.add)
                fbar16s.append(fb)

            # ---- interp per piece: res[p, 9l+k, c] = sum_j h_j*G[o+k+j,c]
            out_dmas = []
            for pi, (c0, w) in enumerate(PIECES):
                sl = slice(c0, c0 + w)
                # repack gathered records to (slot, col); Act engine except
                # the tail sliver (DVE is faster and Act's sem latency is
                # on the critical path there)
                g_kc = rp.tile([P, USED, w], f16, tag=f"gkc{w}")
                src = g_full[:, sl, 0:USED].transpose([0, 2, 1])
                if pi < len(PIECES) - 1:
                    nc.scalar.copy(out=g_kc[:], in_=src)
                else:
                    nc.vector.tensor_copy(out=g_kc[:], in_=src)
                # dedicated (non-rotating) result tile: the deferred output
                # DMAs below read it after later pieces' MACs have run
                res36 = kp.tile([P, NLVL * K, w], f16, tag=f"res{pi}")
                tmp_t = rp.tile([P, K, w], f16, tag=f"tmp{w}")
                for lvl in range(NLVL):
                    taps, off = TAPS[lvl], OFF[lvl]
                    dst = res36[:, K * lvl : K * (lvl + 1), :]
                    for j in range(taps):
                        gj = g_kc[:, off + j : off + j + K, :]
                        if lvl < 2:
                            hb = (h_ts[lvl][:, j : j + 1, sl]
                                  .to_broadcast([P, K, w]))
                        elif j == 0:
                            hb = (fbar16s[lvl - 2][:, None, sl]
                                  .to_broadcast([P, K, w]))
                        else:
                            hb = (a16s[lvl][:, None, sl]
                                  .to_broadcast([P, K, w]))
                        if j == 0:
                            nc.vector.tensor_tensor(out=dst, in0=gj, in1=hb,
                                                    op=Alu.mult)
                        else:
                            nc.vector.tensor_tensor(out=tmp_t[:], in0=gj,
                                                    in1=hb, op=Alu.mult)
                            nc.vector.tensor_tensor(out=dst, in0=dst,
                                                    in1=tmp_t[:], op=Alu.add)
                out_dmas.append((c0, w, res36))

            # output DMAs ride the GpSimd SWDGE path: Pool's in-order
            # stream holds them until every gather is issued, so the out
            # packets never starve gather packets on the shared SDMA
            # engines mid-phase (the recurring ~4-9 us round stall
            # co-occurred with the first piece's output DMA)
            for c0, w, res36 in out_dmas:
                nc.gpsimd.dma_start(
                    out=outd[:, NLVL * K * c0 : NLVL * K * (c0 + w)]
                    .rearrange("p (a b) -> p a b", a=NLVL * K),
                    in_=res36[:])

    return nc


def _prep_core(corrs_core, n_pix):
    """Merged-record table [n_pix*NREC, REC] f16 for one core."""
    from numpy.lib.stride_tricks import sliding_window_view as swv

    recs = np.zeros((n_pix, NREC, REC), dtype=np.float16)
    for i in range(NLVL):
        wi = WS[i]
        padded = np.zeros((n_pix, 4 + wi + 10), dtype=np.float32)
        padded[:, 4 : 4 + wi] = corrs_core[i]
        win = swv(padded, SLC[i], axis=1)
        if i < 3:
            stride = 4 >> i
            win = win[:, ::stride][:, :NREC]
        else:
            win = win[:, np.arange(NREC) >> 1]
        recs[:, :, OFF[i] : OFF[i] + SLC[i]] = win
    return recs.reshape(n_pix * NREC, REC)


def _static_tables(n_pix):
    """Host-precomputed iota tables (data-independent)."""
    mwc = n_pix // 16 // 4
    m = np.arange(mwc, dtype=np.int32)
    c = np.arange(P, dtype=np.int32)
    base = (1024 * (m[None, :] % 32) + 64 * (c[:, None] % 16)).astype(np.int16)
    jf = np.broadcast_to(
        np.arange(TAPS[0], dtype=np.float16)[None, :, None],
        (P, TAPS[0], n_pix // P),
    ).copy()
    return base, jf


_CACHE = {}


def kernel(corr0, corr1, corr2, corr3, flow):
    """Full-input entry point: shard over 8 cores, run, gather."""
    from concourse.bass_utils import run_bass_kernel_spmd

    n_cores = 8
    n_pix = B * H * W // n_cores
    tcol = n_pix // P
    qcol = tcol // NQ

    if "nc" not in _CACHE:
        nc = build_bass(n_pix=n_pix)
        nc.finalize()
        _CACHE["nc"] = nc
    nc = _CACHE["nc"]

    corrs = [
        np.asarray(c, dtype=np.float32).reshape(B * H * W, w)
        for c, w in zip((corr0, corr1, corr2, corr3), WS)
    ]
    flow = np.asarray(flow, dtype=np.float32)
    disp_full = flow[:, 0].reshape(B * H * W)
    base16, jf16 = _static_tables(n_pix)

    in_maps = []
    for c in range(n_cores):
        sl = slice(c * n_pix, (c + 1) * n_pix)
        disp = np.ascontiguousarray(disp_full[sl])
        dw16 = disp.reshape(n_pix // 16, 16).T          # (16, mw)
        in_maps.append({
            "rec": _prep_core([cr[sl] for cr in corrs], n_pix),
            "disp_cm": np.ascontiguousarray(disp.reshape(tcol, P).T),
            "disp_w": np.ascontiguousarray(np.tile(dw16, (8, 1))),
            "base16": base16,
            "jf16": jf16,
        })

    res = run_bass_kernel_spmd(nc, in_maps, list(range(n_cores)),
                               trace=_CACHE.get("trace", False))
    _CACHE["last_res"] = res
    pieces = [(0, 48), (48, 48), (96, 48), (144, 12), (156, 12), (168, 12),
              (180, 12)]
    outs = []
    for c in range(n_cores):
        od = res.results[c]["outd"].reshape(P, NLVL * K * tcol)
        # piece (c0, w) holds [36, w] at flat offset 36*c0;
        # out[ch, n] with n = col*128 + p
        blks = [
            od[:, NLVL * K * c0 : NLVL * K * (c0 + w)]
            .reshape(P, NLVL * K, w).transpose(1, 2, 0)
            for c0, w in pieces
        ]
        oc = np.concatenate(blks, axis=1).reshape(NLVL * K, n_pix)
        outs.append(oc.reshape(NLVL * K, H, W))
    return np.stack(outs, axis=0).astype(np.float32)
